# revision 53
# baseline (speedup 1.0000x reference)
"""Trainium2 Bass kernel for ContentMultiheadAttention.

Reference computation (L=512, B=32, E=1024, H=16, hd=64):
  q,k,v = x @ W{q,k,v}.T + b    (torch F.linear convention)
  split heads -> [B*H, L, 64]; q /= 8
  S = q @ k.T;  S[mask] = -1e9;  P = softmax(S)
  O = P @ v -> merge heads -> out = O @ Wo.T + bo

Strategy: data-parallel over B across 8 cores (4 graphs/core). Per graph,
scores run in S^T layout ([k, q]) so P^T (exp * binary keep-mask, exact
zeros; scores are bounded so no max-subtraction) is in SBUF with keys on
partitions. The PV matmul uses P^T chunks as the stationary operand and
V (keys on partitions, with a ones-column appended) as the moving
operand, producing O in [q, hd] layout as ap-65 matmuls — this costs
65 cycles per (q-chunk, k-chunk) instead of 512, halving PV PE time,
and the softmax denominator lands as a per-partition (per-query) scalar
so normalization is one DVE reciprocal + tensor_scalar multiply per
head-chunk (no partition broadcast). O is transposed back to [e, q]
for the out-projection by the DMA xbar engine (free on PE). The
out-projection computes out^T (e_out on partitions) so the output bias
is a per-partition ACT bias — no bias matmuls; the host transposes the
stored out^T. The emission is a cross-graph software pipeline: in-proj
of graph b+1 and out-proj of graph b-1 are woven between the attention
head pairs of graph b to keep TensorE fed (ACT owns the exp stream).

Dtypes: the three in-projections run as fp8e4m3 DoubleRow matmuls on a
host-precomputed hi/lo split of both x and W (3-term product hh+hl+lh;
the pair carries ~9 mantissa bits so accuracy exceeds bf16) — DoubleRow
contracts 256/instr at 0.5 cycles/row, cutting in-proj PE time by 25%.
Host pre-scales (x*4, W*32) keep the lo residuals above the e4m3
subnormal floor; the scales ride through qt/kt/vx and are undone for
free in the exp ACT scale (1/16384) and the out-proj ACT scale (1/128).
Scores/PV/out-proj matmuls and all intermediates are fp16 (not bf16:
everything is range-bounded and fp16's 10 mantissa bits halve the
rounding error at identical cost). PSUM accumulation is fp32.
"""

import numpy as np
import ml_dtypes

import concourse.mybir as mybir
import concourse.tile as tile
from concourse import bacc
from concourse import bass_utils

L, B, E, H = 512, 32, 1024, 16
HD = E // H  # 64
NCORES = 8
BPC = B // NCORES  # graphs per core

BF = mybir.dt.bfloat16
F16 = mybir.dt.float16
FP8 = mybir.dt.float8e4
F32 = mybir.dt.float32
AF = mybir.ActivationFunctionType
ALU = mybir.AluOpType
DR = mybir.MatmulPerfMode.DoubleRow

# Host pre-scales for the fp8 hi/lo split (undone by ACT scale=1/(XS*WS)).
# W entries are tiny (sigma ~0.022); scaling keeps the lo residual above the
# e4m3 subnormal floor (2^-9).
XS = 4.0
WS = 32.0
INV_S = 1.0 / (XS * WS)

_BUILT = {}


def _build_module():
    """Construct + compile the per-core Bacc program (same NEFF on all cores)."""
    nc = bacc.Bacc(None, target_bir_lowering=False, debug=False)

    # --- DRAM I/O (per core) ---
    # x*: [graph, p, hi/lo, ein_chunk, token] — fp8 hi/lo pair of X^T·XS.
    # The in-proj runs as 3-term fp8 DoubleRow products (hh + lo·hi cross
    # terms); the hi/lo pair carries ~9 mantissa bits so accuracy >= bf16.
    xq = nc.dram_tensor("xq", [BPC, 128, 2, 8, L], FP8, kind="ExternalInput").ap()
    xk = nc.dram_tensor("xk", [BPC, 128, 2, 8, L], FP8, kind="ExternalInput").ap()
    xv = nc.dram_tensor("xv", [BPC, 128, 2, 8, L], FP8, kind="ExternalInput").ap()
    # mask^T as multiplicative binary (1=keep, 0=masked): [graph, p, kc, q]
    mneg = nc.dram_tensor("mneg", [BPC, 128, 4, L], F16, kind="ExternalInput").ap()
    # Wq/Wk^T·WS fp8 hi/lo, sliced by e_out chunk: [eo, p, hi/lo, ei, col]
    wq = nc.dram_tensor("wq", [8, 128, 2, 8, 128], FP8, kind="ExternalInput").ap()
    wk = nc.dram_tensor("wk", [8, 128, 2, 8, 128], FP8, kind="ExternalInput").ap()
    # Wv^T·WS fp8 hi/lo: [p, hi/lo, ein_chunk, e_out]
    wv = nc.dram_tensor("wv", [128, 2, 8, E], FP8, kind="ExternalInput").ap()
    # Wo^T·WS fp8 hi/lo: [p, hi/lo, ein_chunk, e_out] (pair-major like oat)
    wo = nc.dram_tensor("wo", [128, 2, 8, E], FP8, kind="ExternalInput").ap()
    # q/k biases per e_out partition: [p, eo_chunk]
    bq = nc.dram_tensor("bq", [128, 8], F32, kind="ExternalInput").ap()
    bk = nc.dram_tensor("bk", [128, 8], F32, kind="ExternalInput").ap()
    # effective output bias (bo + Wo @ bv) per e_out partition: [p, eoc]
    bo2 = nc.dram_tensor("bo2", [128, 8], F32, kind="ExternalInput").ap()
    # out^T tiles: [graph, eo_chunk, p, token]; host transposes back.
    # bf16 keeps the final copy single-wait (8 dedicated f_sb buffers fit)
    # and halves store traffic; host converts to fp32.
    out = nc.dram_tensor("out", [BPC, 8, 128, L], F16, kind="ExternalOutput").ap()

    with tile.TileContext(nc) as tc:
        with (
            tc.tile_pool(name="wpool", bufs=1) as wpool,
            tc.tile_pool(name="xpool", bufs=1) as xpool,
            tc.tile_pool(name="gpool", bufs=2) as gpool,
            tc.tile_pool(name="spool", bufs=3) as spool,
            tc.tile_pool(name="ppsum", bufs=2, space="PSUM") as ppsum,
            tc.tile_pool(name="spsum", bufs=2, space="PSUM") as spsum,
            tc.tile_pool(name="opsum", bufs=2, space="PSUM") as opsum,
        ):
            # resident weights. wq/wk are eo-sliced [p, eo, hl, ei, col] so
            # the first in-proj matmuls can start after one slice + x-hi.
            wq_sb = wpool.tile([128, 8, 2, 8, 128], FP8)
            wk_sb = wpool.tile([128, 8, 2, 8, 128], FP8)
            wv_sb = wpool.tile([128, 2, 8, E], FP8)
            wo_sb = wpool.tile([128, 2, 8, E], FP8)
            bq_sb = wpool.tile([128, 8], F32)
            bk_sb = wpool.tile([128, 8], F32)
            bo2_sb = wpool.tile([128, 8], F32)

            def load_graph(b):
                st = {}
                st["xq"] = xpool.tile([128, 2, 8, L], FP8, tag="xq", name="xq")
                st["xk"] = xpool.tile([128, 2, 8, L], FP8, tag="xk", name="xk")
                st["xv"] = xpool.tile([128, 2, 8, L], FP8, tag="xv", name="xv")
                st["mneg"] = xpool.tile([128, 4, L], F16, tag="mneg", bufs=2, name="mneg")
                if b == 0:
                    # startup-ordered loads: x-hi + W-hi slices first so the
                    # hh matmuls of the first QT psum groups start ASAP; lo
                    # parts stream while hh runs (group order is hh, lh, hl).
                    nc.sync.dma_start(st["xq"][:, 0:1, 0:2, :], xq[b, :, 0:1, 0:2, :])
                    nc.sync.dma_start(wq_sb[:, 0, 0:1], wq[0, :, 0:1])
                    nc.sync.dma_start(bq_sb[:], bq[:])
                    nc.sync.dma_start(st["xq"][:, 0:1, 2:8, :], xq[b, :, 0:1, 2:8, :])
                    nc.sync.dma_start(st["xq"][:, 1:2, :, :], xq[b, :, 1:2, :, :])
                    nc.sync.dma_start(wq_sb[:, 0, 1:2], wq[0, :, 1:2])
                    for eo in range(1, 8):
                        nc.sync.dma_start(wq_sb[:, eo], wq[eo])
                    nc.sync.dma_start(st["xk"][:, 0:1, :, :], xk[b, :, 0:1, :, :])
                    nc.sync.dma_start(wk_sb[:, 0], wk[0])
                    nc.sync.dma_start(bk_sb[:], bk[:])
                    nc.sync.dma_start(st["xk"][:, 1:2, :, :], xk[b, :, 1:2, :, :])
                    for eo in range(1, 8):
                        nc.sync.dma_start(wk_sb[:, eo], wk[eo])
                    nc.sync.dma_start(st["xv"][:], xv[b])
                    nc.sync.dma_start(wv_sb[:], wv[:])
                    nc.sync.dma_start(st["mneg"][:], mneg[b])
                    nc.sync.dma_start(wo_sb[:], wo[:])
                    nc.sync.dma_start(bo2_sb[:], bo2[:])
                else:
                    nc.sync.dma_start(st["xq"][:], xq[b])
                    nc.sync.dma_start(st["xk"][:], xk[b])
                    nc.sync.dma_start(st["xv"][:], xv[b])
                    nc.sync.dma_start(st["mneg"][:], mneg[b])
                st["qt"] = gpool.tile([128, 8, L], F16, tag="qt", name="qt")
                st["kt"] = gpool.tile([128, 8, L], F16, tag="kt", name="kt")
                st["vx"] = gpool.tile([128, 4, H, HD + 1], F16, tag="vx", name="vx")
                st["osb"] = gpool.tile([128, 4, E], F16, tag="osb", bufs=1,
                                       name="osb")
                st["oat"] = gpool.tile([128, 8, L], F16, tag="oat", bufs=4,
                                       name="oat")
                nc.vector.memset(st["vx"][:, :, :, HD], 8.0)
                return st

            # 3-term hi/lo product order: hh, lh (x-lo), hl (w-lo). Each term
            # runs as 4 fp8 DoubleRow matmuls over ei-chunk pairs (K=256 per
            # instr at 0.5 cycles/row -> 12*256 rows vs bf16's 8*512).
            HL_TERMS = ((0, 0), (0, 1), (1, 0))  # (w hi/lo, x hi/lo)

            def inproj_pieces(st):
                """24 emit-closures: 16 QT/KT psum groups + 8 V groups."""
                pieces = []
                for w_sb, xkey, dkey, bias_sb in (
                    (wq_sb, "xq", "qt", bq_sb),
                    (wk_sb, "xk", "kt", bk_sb),
                ):
                    for eo in range(8):
                        def qk_piece(w_sb=w_sb, xkey=xkey, dkey=dkey,
                                     bias_sb=bias_sb, eo=eo):
                            ps = ppsum.tile([128, 512], F32, tag="ppsum")
                            idx = 0
                            for whl, xhl in HL_TERMS:
                                for jp in range(4):
                                    nc.tensor.matmul(
                                        ps[:],
                                        w_sb[:, eo, whl, 2 * jp : 2 * jp + 2, :],
                                        st[xkey][:, xhl, 2 * jp : 2 * jp + 2, :],
                                        start=(idx == 0),
                                        stop=(idx == 11),
                                        perf_mode=DR,
                                    )
                                    idx += 1
                            # qt/kt keep the XS*WS=128 host scale, undone
                            # in the exp's ACT scale; bias host-prescaled.
                            nc.scalar.activation(
                                st[dkey][:, eo, :], ps[:], AF.Identity,
                                bias=bias_sb[:, eo : eo + 1], scale=1.0,
                            )
                        pieces.append(qk_piece)
                for t4 in range(4):
                    for ec in range(2):
                        def v_piece(t4=t4, ec=ec):
                            ps = ppsum.tile([128, 512], F32, tag="ppsum")
                            idx = 0
                            for whl, xhl in HL_TERMS:
                                for jp in range(4):
                                    nc.tensor.matmul(
                                        ps[:],
                                        st["xv"][:, xhl, 2 * jp : 2 * jp + 2,
                                                 t4 * 128 : (t4 + 1) * 128],
                                        wv_sb[:, whl, 2 * jp : 2 * jp + 2,
                                              ec * 512 : (ec + 1) * 512],
                                        start=(idx == 0),
                                        stop=(idx == 11),
                                        perf_mode=DR,
                                    )
                                    idx += 1
                            # vx keeps the 128x host scale; undone in the
                            # final out-proj ACT scale.
                            nc.scalar.activation(
                                st["vx"][:, t4, ec * 8 : (ec + 1) * 8, 0:HD],
                                ps.rearrange("p (h d) -> p h d", d=HD),
                                AF.Copy,
                            )
                        pieces.append(v_piece)
                return pieces

            def emit_scores(st, hp, pts=None, upto=4):
                """Score pair (par0|par1) lands in one 2-bank psum tile so a
                single fused [128,1024] exp covers both heads of the pair.
                Emitted in two halves (kc<2, kc>=2) so other PE work can sit
                between them while exp frees the psum pool."""
                if pts is None:
                    pts = []
                for kc in range(len(pts), upto):
                    sps = spsum.tile([128, 2, 512], F32, tag="spsum",
                                     padded_shape=[128, 2, 512])
                    for par in (0, 1):
                        po = par * 64
                        nc.tensor.matmul(
                            sps[:, par, :],
                            st["kt"][po : po + 64, hp, kc * 128 : (kc + 1) * 128],
                            st["qt"][po : po + 64, hp, :],
                            start=True,
                            stop=True,
                        )
                    pt = spool.tile([128, 2, 512], F16, tag="pt", bufs=8)
                    # qt/kt carry 128x each -> scores psum is 16384x
                    nc.scalar.activation(pt[:], sps[:], AF.Exp,
                                         scale=1.0 / (XS * WS) ** 2)
                    # zero masked entries (fp16 SBUF multiply)
                    for par in (0, 1):
                        nc.vector.tensor_tensor(
                            pt[:, par, :], pt[:, par, :], st["mneg"][:, kc, :],
                            op=ALU.mult,
                        )
                    pts.append(pt)
                return pts


            def emit_pv_mms(st, hp, pts):
                """P^T chunks stationary, V(+ones) moving -> O [q, hd(+1)]."""
                tiles = []
                for par in (0, 1):
                    h = 2 * hp + par
                    ops = opsum.tile([128, 4 * (HD + 1)], F32, tag="opsum",
                                     padded_shape=[128, 512])
                    for qc in range(4):
                        sl = slice(qc * 65, qc * 65 + 65)
                        for kc in range(4):
                            nc.tensor.matmul(
                                ops[:, sl],
                                pts[kc][:, par, qc * 128 : (qc + 1) * 128],
                                st["vx"][:, kc, h, :],
                                start=(kc == 0),
                                stop=(kc == 3),
                            )
                    tiles.append(ops)
                return tiles

            def emit_pv_norm(st, hp, tiles, last=False):
                """Denominator is psum column HD of each 65-block; normalize
                is a per-partition reciprocal + tensor_scalar into osb.
                Emitted after the next pair's mask multiplies so the DVE mask
                path is never queued behind the PV-dependent normalize."""
                # osb holds a PACKED fp8 hi/lo pair per fp16 slot (hi at
                # byte 0, lo at byte 1): osb = 16*attn (ones column is 8.0
                # and vx carries 128x, so 128/8 = 16) fits fp8e4 range. The
                # 2-byte DMA transpose then moves both halves at once and
                # the DoubleRow out-proj reads them via byte-strided APs --
                # no post-transpose split ops at all.
                osb8 = st["osb"][:, :, :].bitcast(FP8).rearrange(
                    "p qc (e two) -> p qc e two", two=2)
                for par in (0, 1):
                    h = 2 * hp + par
                    ops = tiles[par]
                    rcp = spool.tile([128, 4], F32, tag="rcp", bufs=4)
                    denoms = ops.rearrange("p (qc u) -> p qc u", u=65)[:, :, HD]
                    nc.vector.reciprocal(rcp[:], denoms)
                    for qc in range(4):
                        hi = osb8[:, qc, h * HD : (h + 1) * HD, 0]
                        lo = osb8[:, qc, h * HD : (h + 1) * HD, 1]
                        if par == 0:
                            # ACT applies the per-query reciprocal natively
                            # (per-partition scale AP), halving the DVE load
                            nc.scalar.activation(
                                hi, ops[:, qc * 65 : qc * 65 + HD],
                                AF.Copy, scale=rcp[:, qc : qc + 1],
                            )
                        else:
                            nc.vector.tensor_scalar_mul(
                                hi,
                                ops[:, qc * 65 : qc * 65 + HD],
                                rcp[:, qc : qc + 1],
                            )
                        nc.vector.scalar_tensor_tensor(
                            lo,
                            ops[:, qc * 65 : qc * 65 + HD],
                            rcp[:, qc : qc + 1], hi,
                            op0=ALU.mult, op1=ALU.subtract,
                        )
                # O [q, e]-range -> oat [e, q] via DMA xbar transpose; out
                # chunk c of [128, C, 128] holds transposed rows e = c*128+p
                # (pair-major, matches oat). Pairs 0-3 go after hp 3, pairs
                # 4-7 after hp 7.
                spans = {3: (0, 4), 7: (4, 8)}
                if hp in spans:
                    p0, p1 = spans[hp]
                    for qc in range(4):
                        nc.sync.dma_start(
                            st["oat"][:, p0:p1, qc * 128 : (qc + 1) * 128],
                            st["osb"][:, qc, p0 * 128 : p1 * 128],
                            transpose=True,
                        )
                return []

            def emit_attention(st, pieces, last=False, carry=0):
                """Head pairs, PV one pair behind scores, in-proj pieces of
                the NEXT graph woven between pairs to keep PE fed while ACT
                runs the exp stream."""
                # On the last graph, hold back a few pieces to run after the
                # final PV so PE stays busy while the last transposes drain.
                reserve = 3 if last else carry
                split_q = []
                prev = None
                for hp in range(8):
                    pts = emit_scores(st, hp, upto=2)
                    if prev is not None:
                        # PV matmuls of the previous pair sit between score
                        # kc chunks so PE has ready work while the exp stream
                        # frees the 2-buffer score psum pool (PE is in-order).
                        tiles = emit_pv_mms(st, hp - 1, prev)
                    emit_scores(st, hp, pts=pts, upto=4)
                    if prev is not None:
                        new_splits = emit_pv_norm(st, hp - 1, tiles,
                                                  last=last)
                        # one-span delay: by now the previous span's
                        # transposes have landed, so its split ops wait ~0 at
                        # the ACT/DVE queue heads
                        if new_splits and split_q:
                            split_q.pop(0)()
                        split_q += new_splits
                    n_pop = (len(pieces) - reserve + (7 - hp)) // (8 - hp)
                    for _ in range(max(0, n_pop)):
                        if len(pieces) > reserve:
                            pieces.pop(0)()
                    prev = pts
                tiles = emit_pv_mms(st, 7, prev)
                split_q += emit_pv_norm(st, 7, tiles, last=last)
                if not last:
                    # span(2,4)... splits whose transposes are old flush now;
                    # the freshest span's split rolls into the next phase
                    while len(split_q) > 1:
                        split_q.pop(0)()
                    # Leave a couple of pieces for the next phase: they pop
                    # AFTER the next phase's first scores+exp are emitted, so
                    # the ACT/DVE backlog at the boundary never stalls PE.
                    rest, pieces = pieces[:-carry] if carry else pieces, \
                        pieces[-carry:] if carry else []
                    for p in rest:
                        p()
                    return split_q + pieces
                while split_q:
                    split_q.pop(0)()
                while pieces:
                    pieces.pop(0)()
                return []

            def outproj_pieces(st, b, final=False):
                """out^T tiles [e_out chunk, tokens]; bias is per-partition.
                The very last piece splits its copy+store into chunks so the
                final DMA chain drains sooner after the last matmul."""
                pieces = []
                for eoc in range(8):
                    def o_piece(eoc=eoc):
                        fps = ppsum.tile([128, 512], F32, tag="ppsum")
                        f_sb = spool.tile([128, 512], F16, tag="fsb", bufs=7)
                        oat8 = st["oat"][:, :, :].bitcast(FP8).rearrange(
                            "p hp (q two) -> p hp q two", two=2)
                        idx = 0
                        for whl, ohl in ((0, 0), (1, 0), (0, 1)):
                            for jp in range(4):
                                nc.tensor.matmul(
                                    fps[:],
                                    wo_sb[:, whl, 2 * jp : 2 * jp + 2,
                                          eoc * 128 : (eoc + 1) * 128],
                                    oat8[:, 2 * jp : 2 * jp + 2, :, ohl],
                                    start=(idx == 0),
                                    stop=(idx == 11),
                                    perf_mode=DR,
                                )
                                idx += 1
                        # stores go out on the otherwise-idle GPSIMD (SWDGE)
                        # queue so their copy-waits never clog SP.SEQ, which
                        # carries the transposes and input loads.
                        if final and eoc >= 5:
                            # tail stores ride the (idle) ACT hwdge queue in
                            # natural order right behind their copies
                            nc.scalar.activation(
                                f_sb[:], fps[:], AF.Identity,
                                bias=bo2_sb[:, eoc : eoc + 1],
                                scale=1.0 / 512.0,
                            )
                            nc.scalar.dma_start(out[b, eoc], f_sb[:])
                        else:
                            nc.scalar.activation(
                                f_sb[:], fps[:], AF.Identity,
                                bias=bo2_sb[:, eoc : eoc + 1],
                                scale=1.0 / 512.0,
                            )
                            nc.gpsimd.dma_start(out[b, eoc], f_sb[:])
                    pieces.append(o_piece)
                return pieces

            # Weave plan (keeps every attention phase PE-bound vs the ACT
            # exp stream): att(0): inproj(1); att(1): inproj(2)+op(0)[:4];
            # att(2): inproj(3); att(3): op(0)[4:]+op(1)+op(2) (20 pieces so
            # the final attention phase stays PE-bound too; oat bufs=4).
            st = load_graph(0)
            for p in inproj_pieces(st):
                p()
            states = [st]
            deferred = []
            for b in range(1, BPC):
                st_next = load_graph(b)
                pieces = inproj_pieces(st_next)
                if b == 2:
                    deferred += outproj_pieces(states[0], 0)
                elif b == 3:
                    deferred += outproj_pieces(states[1], 1)
                emit_attention(states[b - 1], pieces)
                states.append(st_next)
            emit_attention(
                states[BPC - 1],
                deferred + outproj_pieces(states[BPC - 2], BPC - 2),
                last=True,
            )
            for p in outproj_pieces(states[BPC - 1], BPC - 1, final=True):
                p()

    nc.compile()
    return nc


# revision 54
# speedup vs baseline: 1.1102x; 1.1102x over previous
"""Trainium2 Bass kernel for ContentMultiheadAttention.

Reference computation (L=512, B=32, E=1024, H=16, hd=64):
  q,k,v = x @ W{q,k,v}.T + b    (torch F.linear convention)
  split heads -> [B*H, L, 64]; q /= 8
  S = q @ k.T;  S[mask] = -1e9;  P = softmax(S)
  O = P @ v -> merge heads -> out = O @ Wo.T + bo

Strategy: data-parallel over B across 8 cores (4 graphs/core). Per graph,
scores run in S^T layout ([k, q]) so P^T (exp * binary keep-mask, exact
zeros; scores are bounded so no max-subtraction) is in SBUF with keys on
partitions. The PV matmul uses P^T chunks as the stationary operand and
V (keys on partitions, with a ones-column appended) as the moving
operand, producing O in [q, hd] layout as ap-65 matmuls — this costs
65 cycles per (q-chunk, k-chunk) instead of 512, halving PV PE time,
and the softmax denominator lands as a per-partition (per-query) scalar
so normalization is one DVE reciprocal + tensor_scalar multiply per
head-chunk (no partition broadcast). O is transposed back to [e, q]
for the out-projection by the DMA xbar engine (free on PE). The
out-projection computes out^T (e_out on partitions) so the output bias
is a per-partition ACT bias — no bias matmuls; the host transposes the
stored out^T. The emission is a cross-graph software pipeline: in-proj
of graph b+1 and out-proj of graph b-1 are woven between the attention
head pairs of graph b to keep TensorE fed (ACT owns the exp stream).

Dtypes: the three in-projections run as fp8e4m3 DoubleRow matmuls on a
host-precomputed hi/lo split of both x and W (3-term product hh+hl+lh;
the pair carries ~9 mantissa bits so accuracy exceeds bf16) — DoubleRow
contracts 256/instr at 0.5 cycles/row, cutting in-proj PE time by 25%.
Host pre-scales (x*4, W*32) keep the lo residuals above the e4m3
subnormal floor; the scales ride through qt/kt/vx and are undone for
free in the exp ACT scale (1/16384) and the out-proj ACT scale (1/128).
Scores/PV/out-proj matmuls and all intermediates are fp16 (not bf16:
everything is range-bounded and fp16's 10 mantissa bits halve the
rounding error at identical cost). PSUM accumulation is fp32.
"""

import numpy as np
import ml_dtypes

import concourse.mybir as mybir
import concourse.tile as tile
from concourse import bacc
from concourse import bass_utils

L, B, E, H = 512, 32, 1024, 16
HD = E // H  # 64
NCORES = 8
BPC = B // NCORES  # graphs per core

BF = mybir.dt.bfloat16
F16 = mybir.dt.float16
FP8 = mybir.dt.float8e4
F32 = mybir.dt.float32
AF = mybir.ActivationFunctionType
ALU = mybir.AluOpType
DR = mybir.MatmulPerfMode.DoubleRow

# Host pre-scales for the fp8 hi/lo split (undone by ACT scale=1/(XS*WS)).
# W entries are tiny (sigma ~0.022); scaling keeps the lo residual above the
# e4m3 subnormal floor (2^-9).
XS = 4.0
WS = 32.0
INV_S = 1.0 / (XS * WS)

_BUILT = {}


def _build_module():
    """Construct + compile the per-core Bacc program (same NEFF on all cores)."""
    nc = bacc.Bacc(None, target_bir_lowering=False, debug=False)

    # --- DRAM I/O (per core) ---
    # x*: [graph, p, hi/lo, ein_chunk, token] — fp8 hi/lo pair of X^T·XS.
    # The in-proj runs as 3-term fp8 DoubleRow products (hh + lo·hi cross
    # terms); the hi/lo pair carries ~9 mantissa bits so accuracy >= bf16.
    xq = nc.dram_tensor("xq", [BPC, 128, 2, 8, L], FP8, kind="ExternalInput").ap()
    xk = nc.dram_tensor("xk", [BPC, 128, 2, 8, L], FP8, kind="ExternalInput").ap()
    xv = nc.dram_tensor("xv", [BPC, 128, 2, 8, L], FP8, kind="ExternalInput").ap()
    # mask^T as multiplicative binary (1=keep, 0=masked): [graph, p, kc, q]
    mneg = nc.dram_tensor("mneg", [BPC, 128, 4, L], F16, kind="ExternalInput").ap()
    # Wq/Wk^T·WS fp8 hi/lo, sliced by e_out chunk: [eo, p, hi/lo, ei, col]
    wq = nc.dram_tensor("wq", [8, 128, 2, 8, 128], FP8, kind="ExternalInput").ap()
    wk = nc.dram_tensor("wk", [8, 128, 2, 8, 128], FP8, kind="ExternalInput").ap()
    # Wv^T·WS fp8 hi/lo: [p, hi/lo, ein_chunk, e_out]
    wv = nc.dram_tensor("wv", [128, 2, 8, E], FP8, kind="ExternalInput").ap()
    # Wo^T: [p, ein_chunk, e_out] (chunked pair-major to match oat)
    wo = nc.dram_tensor("wo", [128, 8, E], F16, kind="ExternalInput").ap()
    # q/k biases per e_out partition: [p, eo_chunk]
    bq = nc.dram_tensor("bq", [128, 8], F32, kind="ExternalInput").ap()
    bk = nc.dram_tensor("bk", [128, 8], F32, kind="ExternalInput").ap()
    # effective output bias (bo + Wo @ bv) per e_out partition: [p, eoc]
    bo2 = nc.dram_tensor("bo2", [128, 8], F32, kind="ExternalInput").ap()
    # out^T tiles: [graph, eo_chunk, p, token]; host transposes back.
    # bf16 keeps the final copy single-wait (8 dedicated f_sb buffers fit)
    # and halves store traffic; host converts to fp32.
    out = nc.dram_tensor("out", [BPC, 8, 128, L], F16, kind="ExternalOutput").ap()

    with tile.TileContext(nc) as tc:
        with (
            tc.tile_pool(name="wpool", bufs=1) as wpool,
            tc.tile_pool(name="xpool", bufs=1) as xpool,
            tc.tile_pool(name="gpool", bufs=2) as gpool,
            tc.tile_pool(name="spool", bufs=3) as spool,
            tc.tile_pool(name="ppsum", bufs=2, space="PSUM") as ppsum,
            tc.tile_pool(name="spsum", bufs=2, space="PSUM") as spsum,
            tc.tile_pool(name="opsum", bufs=2, space="PSUM") as opsum,
        ):
            # resident weights. wq/wk are eo-sliced [p, eo, hl, ei, col] so
            # the first in-proj matmuls can start after one slice + x-hi.
            wq_sb = wpool.tile([128, 8, 2, 8, 128], FP8)
            wk_sb = wpool.tile([128, 8, 2, 8, 128], FP8)
            wv_sb = wpool.tile([128, 2, 8, E], FP8)
            wo_sb = wpool.tile([128, 8, E], F16)
            bq_sb = wpool.tile([128, 8], F32)
            bk_sb = wpool.tile([128, 8], F32)
            bo2_sb = wpool.tile([128, 8], F32)

            def load_graph(b):
                st = {}
                st["xq"] = xpool.tile([128, 2, 8, L], FP8, tag="xq", name="xq")
                st["xk"] = xpool.tile([128, 2, 8, L], FP8, tag="xk", name="xk")
                st["xv"] = xpool.tile([128, 2, 8, L], FP8, tag="xv", name="xv")
                st["mneg"] = xpool.tile([128, 4, L], F16, tag="mneg", bufs=2, name="mneg")
                if b == 0:
                    # startup-ordered loads: x-hi + W-hi slices first so the
                    # hh matmuls of the first QT psum groups start ASAP; lo
                    # parts stream while hh runs (group order is hh, lh, hl).
                    nc.sync.dma_start(st["xq"][:, 0:1, 0:2, :], xq[b, :, 0:1, 0:2, :])
                    nc.sync.dma_start(wq_sb[:, 0, 0:1], wq[0, :, 0:1])
                    nc.sync.dma_start(bq_sb[:], bq[:])
                    nc.sync.dma_start(st["xq"][:, 0:1, 2:8, :], xq[b, :, 0:1, 2:8, :])
                    nc.sync.dma_start(st["xq"][:, 1:2, :, :], xq[b, :, 1:2, :, :])
                    nc.sync.dma_start(wq_sb[:, 0, 1:2], wq[0, :, 1:2])
                    for eo in range(1, 8):
                        nc.sync.dma_start(wq_sb[:, eo], wq[eo])
                    nc.sync.dma_start(st["xk"][:, 0:1, :, :], xk[b, :, 0:1, :, :])
                    nc.sync.dma_start(wk_sb[:, 0], wk[0])
                    nc.sync.dma_start(bk_sb[:], bk[:])
                    nc.sync.dma_start(st["xk"][:, 1:2, :, :], xk[b, :, 1:2, :, :])
                    for eo in range(1, 8):
                        nc.sync.dma_start(wk_sb[:, eo], wk[eo])
                    nc.sync.dma_start(st["xv"][:], xv[b])
                    nc.sync.dma_start(wv_sb[:], wv[:])
                    nc.sync.dma_start(st["mneg"][:], mneg[b])
                    nc.sync.dma_start(wo_sb[:], wo[:])
                    nc.sync.dma_start(bo2_sb[:], bo2[:])
                else:
                    nc.sync.dma_start(st["xq"][:], xq[b])
                    nc.sync.dma_start(st["xk"][:], xk[b])
                    nc.sync.dma_start(st["xv"][:], xv[b])
                    nc.sync.dma_start(st["mneg"][:], mneg[b])
                st["qt"] = gpool.tile([128, 8, L], F16, tag="qt", name="qt")
                st["kt"] = gpool.tile([128, 8, L], F16, tag="kt", name="kt")
                st["vx"] = gpool.tile([128, 4, H, HD + 1], F16, tag="vx", name="vx")
                st["osb"] = gpool.tile([128, 4, E], F16, tag="osb", bufs=1,
                                       name="osb")
                st["oat"] = gpool.tile([128, 8, L], F16, tag="oat", bufs=4,
                                       name="oat")
                nc.vector.memset(st["vx"][:, :, :, HD], 1.0)
                return st

            # 3-term hi/lo product order: hh, lh (x-lo), hl (w-lo). Each term
            # runs as 4 fp8 DoubleRow matmuls over ei-chunk pairs (K=256 per
            # instr at 0.5 cycles/row -> 12*256 rows vs bf16's 8*512).
            HL_TERMS = ((0, 0), (0, 1), (1, 0))  # (w hi/lo, x hi/lo)

            def inproj_pieces(st):
                """24 emit-closures: 16 QT/KT psum groups + 8 V groups."""
                pieces = []
                for w_sb, xkey, dkey, bias_sb in (
                    (wq_sb, "xq", "qt", bq_sb),
                    (wk_sb, "xk", "kt", bk_sb),
                ):
                    for eo in range(8):
                        def qk_piece(w_sb=w_sb, xkey=xkey, dkey=dkey,
                                     bias_sb=bias_sb, eo=eo):
                            ps = ppsum.tile([128, 512], F32, tag="ppsum")
                            idx = 0
                            for whl, xhl in HL_TERMS:
                                for jp in range(4):
                                    nc.tensor.matmul(
                                        ps[:],
                                        w_sb[:, eo, whl, 2 * jp : 2 * jp + 2, :],
                                        st[xkey][:, xhl, 2 * jp : 2 * jp + 2, :],
                                        start=(idx == 0),
                                        stop=(idx == 11),
                                        perf_mode=DR,
                                    )
                                    idx += 1
                            # qt/kt keep the XS*WS=128 host scale, undone
                            # in the exp's ACT scale; bias host-prescaled.
                            nc.scalar.activation(
                                st[dkey][:, eo, :], ps[:], AF.Identity,
                                bias=bias_sb[:, eo : eo + 1], scale=1.0,
                            )
                        pieces.append(qk_piece)
                for t4 in range(4):
                    for ec in range(2):
                        def v_piece(t4=t4, ec=ec):
                            ps = ppsum.tile([128, 512], F32, tag="ppsum")
                            idx = 0
                            for whl, xhl in HL_TERMS:
                                for jp in range(4):
                                    nc.tensor.matmul(
                                        ps[:],
                                        st["xv"][:, xhl, 2 * jp : 2 * jp + 2,
                                                 t4 * 128 : (t4 + 1) * 128],
                                        wv_sb[:, whl, 2 * jp : 2 * jp + 2,
                                              ec * 512 : (ec + 1) * 512],
                                        start=(idx == 0),
                                        stop=(idx == 11),
                                        perf_mode=DR,
                                    )
                                    idx += 1
                            # vx keeps the 128x host scale; undone in the
                            # final out-proj ACT scale.
                            nc.scalar.activation(
                                st["vx"][:, t4, ec * 8 : (ec + 1) * 8, 0:HD],
                                ps.rearrange("p (h d) -> p h d", d=HD),
                                AF.Copy,
                            )
                        pieces.append(v_piece)
                return pieces

            def emit_scores(st, hp, pts=None, upto=4):
                """Score pair (par0|par1) lands in one 2-bank psum tile so a
                single fused [128,1024] exp covers both heads of the pair.
                Emitted in two halves (kc<2, kc>=2) so other PE work can sit
                between them while exp frees the psum pool."""
                if pts is None:
                    pts = []
                for kc in range(len(pts), upto):
                    sps = spsum.tile([128, 2, 512], F32, tag="spsum",
                                     padded_shape=[128, 2, 512])
                    for par in (0, 1):
                        po = par * 64
                        nc.tensor.matmul(
                            sps[:, par, :],
                            st["kt"][po : po + 64, hp, kc * 128 : (kc + 1) * 128],
                            st["qt"][po : po + 64, hp, :],
                            start=True,
                            stop=True,
                        )
                    pt = spool.tile([128, 2, 512], F16, tag="pt", bufs=8)
                    # qt/kt carry 128x each -> scores psum is 16384x
                    nc.scalar.activation(pt[:], sps[:], AF.Exp,
                                         scale=1.0 / (XS * WS) ** 2)
                    # zero masked entries (bf16 SBUF multiply)
                    for par in (0, 1):
                        nc.vector.tensor_tensor(
                            pt[:, par, :], pt[:, par, :], st["mneg"][:, kc, :],
                            op=ALU.mult,
                        )
                    pts.append(pt)
                return pts


            def emit_pv_mms(st, hp, pts):
                """P^T chunks stationary, V(+ones) moving -> O [q, hd(+1)]."""
                tiles = []
                for par in (0, 1):
                    h = 2 * hp + par
                    ops = opsum.tile([128, 4 * (HD + 1)], F32, tag="opsum",
                                     padded_shape=[128, 512])
                    for qc in range(4):
                        sl = slice(qc * 65, qc * 65 + 65)
                        for kc in range(4):
                            nc.tensor.matmul(
                                ops[:, sl],
                                pts[kc][:, par, qc * 128 : (qc + 1) * 128],
                                st["vx"][:, kc, h, :],
                                start=(kc == 0),
                                stop=(kc == 3),
                            )
                    tiles.append(ops)
                return tiles

            def emit_pv_norm(st, hp, tiles, last=False):
                """Denominator is psum column HD of each 65-block; normalize
                is a per-partition reciprocal + tensor_scalar into osb.
                Emitted after the next pair's mask multiplies so the DVE mask
                path is never queued behind the PV-dependent normalize."""
                for par in (0, 1):
                    h = 2 * hp + par
                    ops = tiles[par]
                    rcp = spool.tile([128, 4], F32, tag="rcp", bufs=4)
                    denoms = ops.rearrange("p (qc u) -> p qc u", u=65)[:, :, HD]
                    nc.vector.reciprocal(rcp[:], denoms)
                    for qc in range(4):
                        nc.vector.tensor_scalar_mul(
                            st["osb"][:, qc, h * HD : (h + 1) * HD],
                            ops[:, qc * 65 : qc * 65 + HD],
                            rcp[:, qc : qc + 1],
                        )
                # O [q, e]-range -> oat [e, q] via DMA xbar transpose; out
                # chunk c of [128, C, 128] holds transposed rows e = c*128+p
                # (pair-major, matches oat). Pairs 0-3 go after hp 3, pairs
                # 4-7 after hp 7.
                spans = {3: (0, 4), 7: (4, 8)}
                if hp in spans:
                    p0, p1 = spans[hp]
                    for qc in range(4):
                        nc.sync.dma_start(
                            st["oat"][:, p0:p1, qc * 128 : (qc + 1) * 128],
                            st["osb"][:, qc, p0 * 128 : p1 * 128],
                            transpose=True,
                        )
                return []

            def emit_attention(st, pieces, last=False, carry=0):
                """Head pairs, PV one pair behind scores, in-proj pieces of
                the NEXT graph woven between pairs to keep PE fed while ACT
                runs the exp stream."""
                # On the last graph, hold back a few pieces to run after the
                # final PV so PE stays busy while the last transposes drain.
                reserve = 3 if last else carry
                split_q = []
                prev = None
                for hp in range(8):
                    pts = emit_scores(st, hp, upto=2)
                    if prev is not None:
                        # PV matmuls of the previous pair sit between score
                        # kc chunks so PE has ready work while the exp stream
                        # frees the 2-buffer score psum pool (PE is in-order).
                        tiles = emit_pv_mms(st, hp - 1, prev)
                    emit_scores(st, hp, pts=pts, upto=4)
                    if prev is not None:
                        new_splits = emit_pv_norm(st, hp - 1, tiles,
                                                  last=last)
                        # one-span delay: by now the previous span's
                        # transposes have landed, so its split ops wait ~0 at
                        # the ACT/DVE queue heads
                        if new_splits and split_q:
                            split_q.pop(0)()
                        split_q += new_splits
                    n_pop = (len(pieces) - reserve + (7 - hp)) // (8 - hp)
                    for _ in range(max(0, n_pop)):
                        if len(pieces) > reserve:
                            pieces.pop(0)()
                    prev = pts
                tiles = emit_pv_mms(st, 7, prev)
                split_q += emit_pv_norm(st, 7, tiles, last=last)
                if not last:
                    # span(2,4)... splits whose transposes are old flush now;
                    # the freshest span's split rolls into the next phase
                    while len(split_q) > 1:
                        split_q.pop(0)()
                    # Leave a couple of pieces for the next phase: they pop
                    # AFTER the next phase's first scores+exp are emitted, so
                    # the ACT/DVE backlog at the boundary never stalls PE.
                    rest, pieces = pieces[:-carry] if carry else pieces, \
                        pieces[-carry:] if carry else []
                    for p in rest:
                        p()
                    return split_q + pieces
                while split_q:
                    split_q.pop(0)()
                while pieces:
                    pieces.pop(0)()
                return []

            def outproj_pieces(st, b, final=False):
                """out^T tiles [e_out chunk, tokens]; bias is per-partition.
                The very last piece splits its copy+store into chunks so the
                final DMA chain drains sooner after the last matmul."""
                pieces = []
                for eoc in range(8):
                    def o_piece(eoc=eoc):
                        fps = ppsum.tile([128, 512], F32, tag="ppsum")
                        f_sb = spool.tile([128, 512], F16, tag="fsb", bufs=7)
                        for hp in range(8):
                            nc.tensor.matmul(
                                fps[:],
                                wo_sb[:, hp, eoc * 128 : (eoc + 1) * 128],
                                st["oat"][:, hp, :],
                                start=(hp == 0),
                                stop=(hp == 7),
                            )
                        # stores go out on the otherwise-idle GPSIMD (SWDGE)
                        # queue so their copy-waits never clog SP.SEQ, which
                        # carries the transposes and input loads.
                        if final and eoc >= 5:
                            # tail stores ride the (idle) ACT hwdge queue in
                            # natural order right behind their copies
                            nc.scalar.activation(
                                f_sb[:], fps[:], AF.Identity,
                                bias=bo2_sb[:, eoc : eoc + 1],
                                scale=1.0 / 128.0,
                            )
                            nc.scalar.dma_start(out[b, eoc], f_sb[:])
                        else:
                            nc.scalar.activation(
                                f_sb[:], fps[:], AF.Identity,
                                bias=bo2_sb[:, eoc : eoc + 1],
                                scale=1.0 / 128.0,
                            )
                            nc.gpsimd.dma_start(out[b, eoc], f_sb[:])
                    pieces.append(o_piece)
                return pieces

            # Weave plan (keeps every attention phase PE-bound vs the ACT
            # exp stream): att(0): inproj(1); att(1): inproj(2)+op(0)[:4];
            # att(2): inproj(3); att(3): op(0)[4:]+op(1)+op(2) (20 pieces so
            # the final attention phase stays PE-bound too; oat bufs=4).
            st = load_graph(0)
            for p in inproj_pieces(st):
                p()
            states = [st]
            deferred = []
            for b in range(1, BPC):
                st_next = load_graph(b)
                pieces = inproj_pieces(st_next)
                if b == 2:
                    deferred += outproj_pieces(states[0], 0)
                elif b == 3:
                    deferred += outproj_pieces(states[1], 1)
                emit_attention(states[b - 1], pieces)
                states.append(st_next)
            emit_attention(
                states[BPC - 1],
                deferred + outproj_pieces(states[BPC - 2], BPC - 2),
                last=True,
            )
            for p in outproj_pieces(states[BPC - 1], BPC - 1, final=True):
                p()

    nc.compile()
    return nc
